# revision 7
# baseline (speedup 1.0000x reference)
"""nGPT-style causal attention block on 8 TRN2 NeuronCores.

Sharding: core = (batch b, head-group g); b = core // 4, g = core % 4.
Each core handles 1 batch x 4 heads (e-slice of 512 channels) and produces a
partial output P = (attention_out @ WoutN[:, sl].T).T of shape [DIM, SEQ];
the host sums the 4 head-group partials per batch and transposes.

All FLOPs (weight row/col l2-norms, projections, per-head q/k l2-norm,
qk_scale, causal softmax attention, output projection) run on device.
Host does only slicing / transposition / final partial-sum gather.

Matmuls run as float32r (full-rate fp32 path on the PE, ~1.5e-4 rel err).
"""
import numpy as np
from contextlib import ExitStack

import concourse.bacc as bacc
import concourse.tile as tile
from concourse import mybir
from concourse.bass_utils import run_bass_kernel_spmd

DIM = 2048          # model dim (= contraction dim of projections)
SEQ = 2048          # sequence length
B = 2               # batch
HEADS = 16
DH = 128            # head dim
NCORES = 8
HPC = 4             # heads per core
ES = HPC * DH       # 512 channels per core
KT = DIM // 128     # 16 contraction tiles
NCH = SEQ // 512    # 4 sequence chunks of 512
ATT_SCALE = float(DH) ** 0.5

f32 = mybir.dt.float32
f32r = mybir.dt.float32r
AF = mybir.ActivationFunctionType
ALU = mybir.AluOpType


def build_program(stop_after=None):
    do_B = stop_after != "A"
    n_heads = 1 if stop_after == "B1" else HPC
    do_C = stop_after is None

    nc = bacc.Bacc("TRN2", target_bir_lowering=False)

    # ---- per-core DRAM I/O ----
    xT_d = nc.dram_tensor("xT", [DIM, SEQ], f32r, kind="ExternalInput")
    wqT_d = nc.dram_tensor("wqT", [HPC, DIM, DH], f32r, kind="ExternalInput")
    wkT_d = nc.dram_tensor("wkT", [HPC, DIM, DH], f32r, kind="ExternalInput")
    wvT_d = nc.dram_tensor("wvT", [DIM, ES], f32r, kind="ExternalInput")
    wqN_d = nc.dram_tensor("wqN", [HPC, 128, DIM], f32, kind="ExternalInput")
    wkN_d = nc.dram_tensor("wkN", [HPC, 128, DIM], f32, kind="ExternalInput")
    wvN_d = nc.dram_tensor("wvN", [HPC, 128, DIM], f32, kind="ExternalInput")
    woT_d = nc.dram_tensor("woT", [ES, DIM], f32r, kind="ExternalInput")
    qs_d = nc.dram_tensor("qs", [128, HPC], f32, kind="ExternalInput")
    tri_d = nc.dram_tensor("tri", [128, 128], f32, kind="ExternalInput")
    onec_d = nc.dram_tensor("onec", [128, 1], f32r, kind="ExternalInput")
    oner_d = nc.dram_tensor("oner", [1, 128], f32r, kind="ExternalInput")
    out_d = nc.dram_tensor("out", [DIM, SEQ], f32, kind="ExternalOutput")

    with tile.TileContext(nc) as tc:
        with ExitStack() as top:
            consts = top.enter_context(tc.tile_pool(name="consts", bufs=1))
            scr = top.enter_context(tc.tile_pool(name="scr", bufs=1, space="DRAM"))
            v_scr = scr.tile([SEQ, ES], f32r)
            oT_scr = scr.tile([HPC, DH, SEQ], f32r)
            # small persistent tiles
            tri_sb = consts.tile([128, 128], f32)
            qs_sb = consts.tile([128, HPC], f32)
            onec_sb = consts.tile([128, 1], f32r)
            oner_sb = consts.tile([1, 128], f32r)
            inv_s2 = consts.tile([128, HPC], f32)
            wv_rn = consts.tile([128, HPC], f32)
            wq_rn = consts.tile([128, HPC], f32)
            wk_rn = consts.tile([128, HPC], f32)
            tmp_se = consts.tile([128, HPC], f32)
            tmp_s2 = consts.tile([128, HPC], f32)
            ssn = consts.tile([128, HPC], f32)     # weight row sum-sq
            ssq = consts.tile([128, HPC], f32)     # sqrt of the above
            nc.sync.dma_start(out=tri_sb, in_=tri_d[:])
            nc.sync.dma_start(out=qs_sb, in_=qs_d[:])
            nc.sync.dma_start(out=onec_sb, in_=onec_d[:])
            nc.sync.dma_start(out=oner_sb, in_=oner_d[:])

            # inv_s2 = 1 / (qs * DIM)^2
            nc.vector.tensor_scalar_mul(tmp_se, qs_sb, float(DIM))
            nc.vector.tensor_mul(tmp_s2, tmp_se, tmp_se)
            nc.vector.reciprocal(inv_s2, tmp_s2)

            # ---- weight row norms (wq, wk, wv) from natural layouts ----
            with tc.tile_pool(name="natw", bufs=2) as natw:
                dump = consts.tile([128, DIM], f32)
                for w_nat_d, rn_tile in ((wvN_d, wv_rn), (wqN_d, wq_rn),
                                         (wkN_d, wk_rn)):
                    for h in range(HPC):
                        nt = natw.tile([128, DIM], f32, tag="nat")
                        nc.sync.dma_start(out=nt, in_=w_nat_d[h])
                        nc.scalar.activation(dump, nt, AF.Square,
                                             accum_out=ssn[:, h:h + 1])
                    nc.scalar.activation(ssq[:, 0:HPC], ssn[:, 0:HPC], AF.Sqrt)
                    nc.vector.reciprocal(rn_tile[:, 0:HPC], ssq[:, 0:HPC])

            # ---- xT resident ----
            with ExitStack() as xctx:
                xpool = xctx.enter_context(tc.tile_pool(name="xpool", bufs=1))
                xt = xpool.tile([128, KT, SEQ], f32r)
                for k in range(KT):
                    nc.sync.dma_start(out=xt[:, k, :],
                                      in_=xT_d[k * 128:(k + 1) * 128, :])

                # ---- phase A: v natural (all heads), spill to DRAM ----
                with tc.tile_pool(name="phA", bufs=2) as phA, \
                     tc.tile_pool(name="phA_ps", bufs=2, space="PSUM") as phA_ps:
                    wvT_sb = phA.tile([128, KT, ES], f32r, tag="wvT")
                    for k in range(KT):
                        nc.sync.dma_start(out=wvT_sb[:, k, :],
                                          in_=wvT_d[k * 128:(k + 1) * 128, :])
                    for t in range(SEQ // 128):
                        pv = phA_ps.tile([128, ES], f32, tag="pv")
                        for k in range(KT):
                            nc.tensor.matmul(
                                pv, xt[:, k, t * 128:(t + 1) * 128],
                                wvT_sb[:, k, :],
                                start=(k == 0), stop=(k == KT - 1))
                        vsb = phA.tile([128, ES], f32r, tag="vev")
                        nc.scalar.copy(vsb, pv)
                        nc.sync.dma_start(
                            out=v_scr[t * 128:(t + 1) * 128, :], in_=vsb[:])

                # ---- phase B: per head ----
                if do_B:
                  with tc.tile_pool(name="phB", bufs=1) as phB, \
                       tc.tile_pool(name="phBw", bufs=2) as phBw, \
                       tc.tile_pool(name="phBn", bufs=3) as phBn, \
                       tc.tile_pool(name="phBs", bufs=2) as phBs, \
                       tc.tile_pool(name="phBe", bufs=3) as phBe:
                    for h in range(n_heads):
                        wqT_sb = phBw.tile([128, KT, DH], f32r, tag="wT")
                        for k in range(KT):
                            nc.sync.dma_start(out=wqT_sb[:, k, :],
                                              in_=wqT_d[h, k * 128:(k + 1) * 128, :])
                        wkT_sb = phBw.tile([128, KT, DH], f32r, tag="wT")
                        for k in range(KT):
                            nc.sync.dma_start(out=wkT_sb[:, k, :],
                                              in_=wkT_d[h, k * 128:(k + 1) * 128, :])

                        qT = phB.tile([128, SEQ], f32r, tag="qT")
                        kT = phB.tile([128, SEQ], f32r, tag="kT")

                        with tc.tile_pool(name="projps", bufs=2, space="PSUM") as pps, \
                             tc.tile_pool(name="projss", bufs=2, space="PSUM") as sps, \
                             tc.tile_pool(name="projbc", bufs=2, space="PSUM") as bps:
                            for dst, wsb, rn_w, is_q in ((qT, wqT_sb, wq_rn, True),
                                                         (kT, wkT_sb, wk_rn, False)):
                                for c in range(NCH):
                                    sl = slice(c * 512, (c + 1) * 512)
                                    pq = pps.tile([128, 512], f32, tag="pq")
                                    for k in range(KT):
                                        nc.tensor.matmul(
                                            pq, wsb[:, k, :], xt[:, k, sl],
                                            start=(k == 0), stop=(k == KT - 1))
                                    # evict with weight-row-norm fold
                                    nc.scalar.activation(
                                        dst[:, sl], pq, AF.Copy,
                                        scale=rn_w[:, h:h + 1])
                                    # per-column (over dh) l2-norm
                                    sq = phBn.tile([128, 512], f32r, tag="nw")
                                    nc.scalar.activation(
                                        sq, dst[:, sl].bitcast(f32), AF.Square)
                                    pss = sps.tile([1, 512], f32, tag="pss")
                                    nc.tensor.matmul(pss, onec_sb, sq,
                                                     start=True, stop=True)
                                    ss_sb = phBs.tile([1, 512], f32r, tag="sssb")
                                    nc.scalar.copy(ss_sb, pss)
                                    pbc = bps.tile([128, 512], f32, tag="pbc")
                                    nc.tensor.matmul(pbc, oner_sb, ss_sb,
                                                     start=True, stop=True)
                                    sq2 = phBn.tile([128, 512], f32, tag="nw")
                                    if is_q:
                                        # sqrt(ss / s_eff^2) = ||q|| / s_eff
                                        nc.scalar.activation(
                                            sq2, pbc, AF.Sqrt,
                                            scale=inv_s2[:, h:h + 1])
                                    else:
                                        nc.scalar.activation(sq2, pbc, AF.Sqrt)
                                    rn2 = phBn.tile([128, 512], f32, tag="nw")
                                    nc.vector.reciprocal(rn2, sq2)
                                    nc.vector.tensor_mul(
                                        dst[:, sl], dst[:, sl].bitcast(f32), rn2)

                        # reload v for this head: [SEQ, DH] as 16 [128, 128]
                        vh = phB.tile([128, SEQ // 128, DH], f32r, tag="vh")
                        for t in range(SEQ // 128):
                            nc.sync.dma_start(
                                out=vh[:, t, :],
                                in_=v_scr[t * 128:(t + 1) * 128,
                                          h * DH:(h + 1) * DH])

                        oT = phB.tile([128, SEQ], f32r, tag="oT")
                        with tc.tile_pool(name="attps", bufs=2, space="PSUM") as aps, \
                             tc.tile_pool(name="attpo", bufs=2, space="PSUM") as ops_, \
                             tc.tile_pool(name="attsum", bufs=2, space="PSUM") as ssps, \
                             tc.tile_pool(name="attbc", bufs=1, space="PSUM") as bcps:
                            for c in range(NCH):
                                isl = slice(c * 512, (c + 1) * 512)
                                nj = 4 * c + 4
                                po = ops_.tile([128, 512], f32, tag="po")
                                psum = ssps.tile([1, 512], f32, tag="psum")
                                for J in range(nj):
                                    ps = aps.tile([128, 512], f32, tag="ps")
                                    nc.tensor.matmul(
                                        ps, kT[:, J * 128:(J + 1) * 128],
                                        qT[:, isl], start=True, stop=True)
                                    esb = phBe.tile([128, 512], f32r, tag="exp")
                                    nc.scalar.activation(esb, ps, AF.Exp,
                                                         scale=ATT_SCALE)
                                    m = J - 4 * c
                                    if m >= 0:
                                        if m > 0:
                                            nc.vector.memset(
                                                esb[:, 0:m * 128].bitcast(f32), 0.0)
                                        nc.vector.tensor_mul(
                                            esb[:, m * 128:(m + 1) * 128],
                                            esb[:, m * 128:(m + 1) * 128].bitcast(f32),
                                            tri_sb)
                                    nc.tensor.matmul(psum, onec_sb, esb,
                                                     start=(J == 0),
                                                     stop=(J == nj - 1))
                                    nc.tensor.matmul(po, vh[:, J, :], esb,
                                                     start=(J == 0),
                                                     stop=(J == nj - 1))
                                ssum_sb = phBs.tile([1, 512], f32r, tag="sssb")
                                nc.scalar.copy(ssum_sb, psum)
                                pbc2 = bcps.tile([128, 512], f32, tag="pbc2")
                                nc.tensor.matmul(pbc2, oner_sb, ssum_sb,
                                                 start=True, stop=True)
                                rs = phBn.tile([128, 512], f32, tag="nw")
                                nc.vector.reciprocal(rs, pbc2)
                                nc.vector.tensor_mul(oT[:, isl], po, rs)
                        nc.sync.dma_start(out=oT_scr[h], in_=oT[:])

            # ---- phase C: output projection (xT freed) ----
            if do_C:
              with tc.tile_pool(name="phC", bufs=1) as phC, \
                   tc.tile_pool(name="phCe", bufs=3) as phCe, \
                   tc.tile_pool(name="phC_ps", bufs=2, space="PSUM") as phC_ps:
                wo = phC.tile([128, HPC, DIM], f32r)
                for t in range(HPC):
                    nc.sync.dma_start(out=wo[:, t, :],
                                      in_=woT_d[t * 128:(t + 1) * 128, :])
                # wout column norms (free-axis over d) combined with wv row norms
                dump2 = phC.tile([128, DIM], f32)
                sso = consts.tile([128, HPC], f32)
                for t in range(HPC):
                    nc.scalar.activation(dump2, wo[:, t, :].bitcast(f32),
                                         AF.Square, accum_out=sso[:, t:t + 1])
                sso_sq = consts.tile([128, HPC], f32)
                nc.scalar.activation(sso_sq, sso, AF.Sqrt)
                wo_rn = consts.tile([128, HPC], f32)
                nc.vector.reciprocal(wo_rn, sso_sq)
                comb = consts.tile([128, HPC], f32)
                nc.vector.tensor_mul(comb, wo_rn, wv_rn)
                for t in range(HPC):
                    nc.vector.tensor_scalar_mul(
                        wo[:, t, :], wo[:, t, :].bitcast(f32), comb[:, t:t + 1])

                oT_all = phC.tile([128, HPC, SEQ], f32r)
                for h in range(HPC):
                    nc.sync.dma_start(out=oT_all[:, h, :], in_=oT_scr[h])

                for d in range(DIM // 128):
                    for c in range(NCH):
                        pP = phC_ps.tile([128, 512], f32, tag="pP")
                        for t in range(HPC):
                            nc.tensor.matmul(
                                pP, wo[:, t, d * 128:(d + 1) * 128],
                                oT_all[:, t, c * 512:(c + 1) * 512],
                                start=(t == 0), stop=(t == HPC - 1))
                        Psb = phCe.tile([128, 512], f32, tag="Pev")
                        nc.scalar.copy(Psb, pP)
                        nc.sync.dma_start(
                            out=out_d[d * 128:(d + 1) * 128,
                                      c * 512:(c + 1) * 512],
                            in_=Psb[:])

    nc.compile()
    return nc


_CACHE = {}


def _get_program(stop_after=None):
    key = stop_after or "full"
    if key not in _CACHE:
        _CACHE[key] = build_program(stop_after)
    return _CACHE[key]


def _make_in_maps(x, Wq, Wk, Wv, Wout, qk_scale):
    tri = np.triu(np.ones((128, 128), dtype=np.float32))  # valid: i' >= j'
    onec = np.ones((128, 1), dtype=np.float32)
    oner = np.ones((1, 128), dtype=np.float32)
    in_maps = []
    for core in range(NCORES):
        b, g = divmod(core, HPC)
        sl = slice(g * ES, (g + 1) * ES)
        wq = Wq[sl]
        wk = Wk[sl]
        wv = Wv[sl]
        in_maps.append({
            "xT": np.ascontiguousarray(x[b].T),
            "wqT": np.ascontiguousarray(
                wq.T.reshape(DIM, HPC, DH).transpose(1, 0, 2)),
            "wkT": np.ascontiguousarray(
                wk.T.reshape(DIM, HPC, DH).transpose(1, 0, 2)),
            "wvT": np.ascontiguousarray(wv.T),
            "wqN": np.ascontiguousarray(wq.reshape(HPC, 128, DIM)),
            "wkN": np.ascontiguousarray(wk.reshape(HPC, 128, DIM)),
            "wvN": np.ascontiguousarray(wv.reshape(HPC, 128, DIM)),
            "woT": np.ascontiguousarray(Wout[:, sl].T),
            "qs": np.ascontiguousarray(
                qk_scale[sl].reshape(HPC, 128).T),
            "tri": tri,
            "onec": onec,
            "oner": oner,
        })
    return in_maps


def _assemble(results):
    out = np.empty((B, SEQ, DIM), dtype=np.float32)
    for b in range(B):
        acc = results[4 * b]["out"].astype(np.float32).copy()
        for g in range(1, HPC):
            acc += results[4 * b + g]["out"]
        out[b] = acc.T
    return out


def kernel(x, Wq, Wk, Wv, Wout, qk_scale):
    nc = _get_program()
    in_maps = _make_in_maps(x, Wq, Wk, Wv, Wout, qk_scale)
    res = run_bass_kernel_spmd(nc, in_maps, core_ids=list(range(NCORES)))
    return _assemble(res.results)
